# revision 1
# baseline (speedup 1.0000x reference)
"""Trainium2 Bass kernel: 15x15 valid cross-correlation over a 4096x4096 f32
image, plus scalar bias. Output: [4082, 4082].

Strategy
--------
Column-shard the output across 8 NeuronCores (512 output columns each; each
core's input slab carries its own 14-column halo, so no device-side exchange).

Per core, the conv is expressed on the tensor engine as PSUM-accumulated
matmuls with banded stationary matrices:

    y[i, n] = sum_dj sum_k T_dj[k, i] * x[r0 + k, c0 + dj + n]
    T_dj[k, i] = w[k - i, dj]   (0 <= k - i < 15)

i.e. for each of the 15 kernel columns dj, one matmul with the [128, 114]
band matrix T_dj as the stationary operand (contraction over a 128-row input
window) and a dj-shifted [128, 512] slice of the input window as the moving
operand, all 15 accumulating into one PSUM bank. 36 windows of 114 output
rows cover the 4082 rows. float32r runs the PE at 1 cycle/row (vs 4 for
strict fp32).
"""

import sys

import numpy as np

sys.path.insert(0, "/opt/trn_rl_repo")

H = W = 4096
KH = KW = 15
OH = OW = H - KH + 1  # 4082
NCORES = 8
COLS_PER_CORE = 512  # output columns computed per core
IN_COLS = COLS_PER_CORE + KW - 1 + 2  # 528: 14-col halo + 2 pad for alignment
M_TILE = 114  # output rows per PE window (128 - 14)
KDIM = 128  # contraction window (input rows per matmul)
NWIN = (OH + M_TILE - 1) // M_TILE  # 36
SLAB_ROWS = (NWIN - 1) * M_TILE + KDIM  # 4118: last window's top edge + 128
PAD_COLS = (NCORES - 1) * COLS_PER_CORE + IN_COLS  # 4112


def _build_bass(n_reps=1):
    import concourse.mybir as mybir
    from concourse import bacc
    from concourse.tile import TileContext

    f32 = mybir.dt.float32
    f32r = mybir.dt.float32r

    # Bacc (not raw Bass): its finalize() runs move_matmul_waits_to_ldweights
    # + generate_event_semaphores, which legalize Tile's multi-wait
    # instructions for TRN2's 1-wait-per-instruction limit.
    nc = bacc.Bacc()
    xs = nc.declare_dram_parameter("xs", [SLAB_ROWS, IN_COLS], f32r, isOutput=False)
    Tm = nc.declare_dram_parameter("Tm", [KDIM, KW * M_TILE], f32r, isOutput=False)
    bcol = nc.declare_dram_parameter("bcol", [KDIM, 1], f32, isOutput=False)
    y = nc.declare_dram_parameter("y", [OH, COLS_PER_CORE], f32, isOutput=True)

    with TileContext(nc) as tc:
        with (
            tc.tile_pool(name="const", bufs=1) as cpool,
            tc.tile_pool(name="xwin", bufs=4) as xpool,
            tc.tile_pool(name="obuf", bufs=4) as opool,
            tc.tile_pool(name="psum", bufs=6, space="PSUM") as ppool,
        ):
            T_sb = cpool.tile([KDIM, KW * M_TILE], f32r)
            nc.sync.dma_start(T_sb[:], Tm[:, :])
            b_sb = cpool.tile([KDIM, 1], f32)
            nc.sync.dma_start(b_sb[:], bcol[:, :])

            for _rep in range(n_reps):
                for t in range(NWIN):
                    xw = xpool.tile([KDIM, IN_COLS], f32r)
                    nc.sync.dma_start(xw[:], xs[M_TILE * t : M_TILE * t + KDIM, :])
                    ps = ppool.tile([M_TILE, COLS_PER_CORE], f32)
                    for dj in range(KW):
                        nc.tensor.matmul(
                            ps[:, :],
                            lhsT=T_sb[:, dj * M_TILE : (dj + 1) * M_TILE],
                            rhs=xw[:, dj : dj + COLS_PER_CORE],
                            start=(dj == 0),
                            stop=(dj == KW - 1),
                        )
                    ob = opool.tile([M_TILE, COLS_PER_CORE], f32)
                    nc.vector.tensor_scalar_add(ob[:, :], ps[:, :], b_sb[:M_TILE, :])
                    rows = min(M_TILE, OH - M_TILE * t)
                    nc.sync.dma_start(
                        y[M_TILE * t : M_TILE * t + rows, :], ob[:rows, :]
                    )

    # run_bass_kernel_spmd's axon path serializes nc.m directly without
    # finalizing; Bacc needs finalize() -> compile() to legalize waits and
    # allocate registers before the IR hits walrus.
    nc.finalize()
    return nc


def _host_prep(x, w, b):
    x = np.asarray(x, dtype=np.float32)
    w = np.asarray(w, dtype=np.float32)
    b = np.asarray(b, dtype=np.float32)

    x_pad = np.zeros((SLAB_ROWS, PAD_COLS), np.float32)
    x_pad[:H, :W] = x

    T_np = np.zeros((KDIM, KW * M_TILE), np.float32)
    i = np.arange(M_TILE)
    for dj in range(KW):
        for di in range(KH):
            T_np[i + di, dj * M_TILE + i] = w[di, dj]

    bcol_np = np.full((KDIM, 1), b[0], np.float32)

    in_maps = []
    for c in range(NCORES):
        slab = np.ascontiguousarray(
            x_pad[:, COLS_PER_CORE * c : COLS_PER_CORE * c + IN_COLS]
        )
        in_maps.append({"xs": slab, "Tm": T_np, "bcol": bcol_np})
    return in_maps


def _enable_ldw_opt():
    """Flip walrus's hardcoded --enable-ldw-opt=false to true.

    The fused f32r matmul's weight load otherwise serializes with the
    moving-operand stream (~75ns exposed per matmul); with ldw-opt the
    loads overlap. Measured 124-129us -> ~80-90us per iteration with
    bit-identical output (rel err unchanged at 1.239e-4).
    """
    from concourse import bass_utils

    if getattr(bass_utils, "_ldw_opt_patched", False):
        return
    orig = bass_utils.run_command

    def patched(cmd, *a, **kw):
        if isinstance(cmd, list):
            cmd = [
                "--enable-ldw-opt=true" if c == "--enable-ldw-opt=false" else c
                for c in cmd
            ]
        return orig(cmd, *a, **kw)

    bass_utils.run_command = patched
    bass_utils._ldw_opt_patched = True


def run(x, w, b, n_reps=1):
    """Build, run on 8 cores, return full output."""
    from concourse.bass_utils import run_bass_kernel_spmd

    _enable_ldw_opt()
    nc = _build_bass(n_reps=n_reps)
    in_maps = _host_prep(x, w, b)
    res = run_bass_kernel_spmd(nc, in_maps, list(range(NCORES)))
    outs = [np.asarray(res.results[c]["y"]) for c in range(NCORES)]
    full = np.concatenate(outs, axis=1)[:, :OW]
    return full


def time_reps(x, w, b, n_reps, n_calls=4):
    """Per-call wall times (s) for an n_reps-body program."""
    import time

    from concourse import bass2jax

    nc = _build_bass(n_reps=n_reps)
    in_maps = _host_prep(x, w, b)
    times = []
    for _ in range(n_calls):
        t0 = time.time()
        bass2jax.run_bass_via_pjrt(nc, in_maps, n_cores=NCORES)
        times.append(time.time() - t0)
    return times


def kernel(x, w, b):
    return run(x, w, b)

